# revision 1
# baseline (speedup 1.0000x reference)
"""Involution (B=4, C=256, H=W=56, K=7, G=16, reduction=4) on 8 trn2 NeuronCores.

Sharding: 8 shards = (batch b in 0..3) x (h-half in 0..1); each core computes
its [256, 28, 56] output slab from a [256, 34, 62] zero-padded input slab.

Per-core pipeline:
  1. matmul1 (PE, bf16): t = relu(bn(W1 @ x)) with BN folded into W1/b1 on host
  2. per-tap "broadcast matmul" (PE, bf16): for each of the 49 kernel taps,
     lhsT = W2bc[k] (a host-precomputed [65,128] slice of W2 whose columns are
     replicated 16x across each group's channels, row 64 carrying b2) so the
     matmul directly yields the per-pixel kernel values broadcast over the 16
     channels of each group: wbc[k][c, p] in PSUM.
  3. involution accumulation (DVE): acc[c,p] += x[c, p+delta_k] * wbc[k][c,p]
"""

import numpy as np
import ml_dtypes
from contextlib import ExitStack

import concourse.bass as bass
import concourse.bacc as bacc
import concourse.tile as tile
from concourse import mybir
from concourse.bass_utils import run_bass_kernel_spmd

BF16 = ml_dtypes.bfloat16

B, C, H, W = 4, 256, 56, 56
KK, G, PAD = 7, 16, 3
Cr, Cg = 64, 16
EPS = 1e-5
HH = H // 2              # 28 rows per h-half shard
PH, PW = HH + 2 * PAD, W + 2 * PAD   # 34, 62 padded slab dims
NPIX = HH * W            # 1568 output pixels per shard
NCORES = 8

_CACHE = {}

# set by test.py to collect a hardware profile
TRACE = False
LAST_RESULT = None


def _build_nc():
    nc = bacc.Bacc("TRN2", target_bir_lowering=False, debug=False,
                   num_devices=NCORES)

    f32 = mybir.dt.float32
    bf16 = mybir.dt.bfloat16

    x_d = nc.declare_dram_parameter("x", [C, PH, PW], f32, isOutput=False)
    w1t_d = nc.declare_dram_parameter("w1t", [2, 128, Cr], bf16, isOutput=False)
    b1p_d = nc.declare_dram_parameter("b1p", [Cr, 1], f32, isOutput=False)
    w2bc_d = nc.declare_dram_parameter("w2bc", [Cr + 1, 49, 2, 128], bf16,
                                       isOutput=False)
    out_d = nc.declare_dram_parameter("out", [C, HH, W], f32, isOutput=True)

    with tile.TileContext(nc) as tc, ExitStack() as ctx:
        const = ctx.enter_context(tc.tile_pool(name="const", bufs=1))
        xpool = ctx.enter_context(tc.tile_pool(name="x", bufs=1))
        tpool = ctx.enter_context(tc.tile_pool(name="t", bufs=1))

        # constants
        w1t_sb = const.tile([128, 2, Cr], bf16)
        for ch in range(2):
            nc.sync.dma_start(w1t_sb[:, ch, :], w1t_d[ch])
        b1p_sb = const.tile([Cr, 1], f32)
        nc.sync.dma_start(b1p_sb[:], b1p_d[:])
        w2bc_sb = const.tile([Cr + 1, 49, 2, 128], bf16)
        nc.sync.dma_start(w2bc_sb[:], w2bc_d[:])

        # input slabs (already zero-padded on host), one per channel half.
        # xb_e is the bf16 cast; xb_o is the same shifted left one column so
        # odd-j tap windows stay 4B-aligned (keeps DVE tensor_tensor in 2x).
        x_bf16 = []
        x_bf16_odd = []
        for ch in range(2):
            xf = xpool.tile([128, PH, PW], f32, tag=f"xf{ch}")
            nc.sync.dma_start(xf[:], x_d[ch * 128:(ch + 1) * 128])
            xb = xpool.tile([128, PH, PW], bf16, tag=f"xb{ch}")
            nc.vector.tensor_copy(xb[:], xf[:])
            x_bf16.append(xb)
            xo = xpool.tile([128, PH, PW - 2], bf16, tag=f"xo{ch}")
            nc.vector.tensor_copy(xo[:], xf[:, :, 1:PW - 1])
            x_bf16_odd.append(xo)

        # ---- stage 1: t_ext = [relu(W1p @ x + b1p); ones] in bf16 ----
        t_ext = tpool.tile([Cr + 1, NPIX], bf16)
        nc.vector.memset(t_ext[Cr:Cr + 1, :], 1.0)
        with tc.tile_pool(name="psum_t", bufs=2,
                          space=bass.MemorySpace.PSUM) as psum_t:
            NROW_CHUNK = 7          # 7 rows x 56 cols = 392 <= 512 (one bank)
            for q in range(HH // NROW_CHUNK):
                pt = psum_t.tile([Cr, NROW_CHUNK * W], f32)
                for ch in range(2):
                    rhs = x_bf16[ch][:, PAD + q * NROW_CHUNK:
                                     PAD + (q + 1) * NROW_CHUNK, PAD:PAD + W]
                    nc.tensor.matmul(pt[:], w1t_sb[:, ch, :], rhs,
                                     start=(ch == 0), stop=(ch == 1))
                nc.scalar.activation(
                    t_ext[0:Cr, q * NROW_CHUNK * W:(q + 1) * NROW_CHUNK * W],
                    pt[:], mybir.ActivationFunctionType.Relu,
                    bias=b1p_sb[:], scale=1.0)

        # ---- stage 2+3: per-tap broadcast matmul + multiply-accumulate ----
        accp = ctx.enter_context(tc.tile_pool(name="acc", bufs=1))
        wbcp = ctx.enter_context(tc.tile_pool(name="wbc", bufs=3))
        outp = ctx.enter_context(tc.tile_pool(name="outp", bufs=2))
        psum_w = ctx.enter_context(
            tc.tile_pool(name="psum_w", bufs=2, space=bass.MemorySpace.PSUM))

        NACC = 4   # parallel bf16 accumulators (keeps rounding error down)
        MM_CHUNKS = [(0, 512), (512, 512), (1024, 512), (1536, 32)]
        for ch in range(2):
            accs = [accp.tile([128, HH, W], bf16, tag=f"acc{ch}_{m}",
                              name=f"acc{ch}_{m}")
                    for m in range(NACC)]
            for k in range(49):
                pw = psum_w.tile([128, NPIX], f32, tag="pw")
                for (o, n) in MM_CHUNKS:
                    nc.tensor.matmul(pw[:, o:o + n], w2bc_sb[:, k, ch, :],
                                     t_ext[:, o:o + n], start=True, stop=True)
                wbc = wbcp.tile([128, HH, W], bf16, tag="wbc")
                nc.scalar.copy(wbc[:].rearrange("p h w -> p (h w)"), pw[:])
                i, j = k // KK, k % KK
                if j % 2 == 0:
                    xwin = x_bf16[ch][:, i:i + HH, j:j + W]
                else:
                    xwin = x_bf16_odd[ch][:, i:i + HH, j - 1:j - 1 + W]
                m = k % NACC
                if k < NACC:
                    nc.vector.tensor_mul(accs[k][:], xwin, wbc[:])
                else:
                    nc.vector.tensor_mul(wbc[:], xwin, wbc[:])
                    if m == 3 or (m == 1 and k >= 21):
                        nc.gpsimd.tensor_add(accs[m][:], accs[m][:], wbc[:])
                    else:
                        nc.vector.tensor_add(accs[m][:], accs[m][:], wbc[:])
            s0 = wbcp.tile([128, HH, W], bf16, tag="wbc")
            nc.vector.tensor_add(s0[:], accs[0][:], accs[1][:])
            s1 = wbcp.tile([128, HH, W], bf16, tag="wbc")
            nc.vector.tensor_add(s1[:], accs[2][:], accs[3][:])
            of = outp.tile([128, HH, W], f32, tag="of")
            nc.vector.tensor_add(of[:], s0[:], s1[:])
            nc.sync.dma_start(out_d[ch * 128:(ch + 1) * 128], of[:])

    nc.compile()
    return nc


def _prep_host_inputs(inputs, W1, b1, gamma, beta, mean, var, W2, b2):
    """Fold BN into W1/b1; build per-tap channel-broadcast W2 slices."""
    scale = gamma / np.sqrt(var + EPS)
    shift = beta - mean * scale
    W1p = W1 * scale[:, None]
    b1p = (b1 * scale + shift).astype(np.float32).reshape(Cr, 1)
    w1t = np.ascontiguousarray(W1p.T.reshape(2, 128, Cr)).astype(BF16)

    O = ((np.arange(2)[None, :, None] * 8
          + np.arange(128)[None, None, :] // 16) * 49
         + np.arange(49)[:, None, None])          # [49, 2, 128]
    w2bc = np.zeros((Cr + 1, 49, 2, 128), np.float32)
    w2bc[0:Cr] = np.transpose(W2[O, :], (3, 0, 1, 2))
    w2bc[Cr] = b2[O]
    w2bc = w2bc.astype(BF16)

    # per-core padded input slabs
    xs = []
    for core in range(NCORES):
        b, hf = core // 2, core % 2
        slab = np.zeros((C, PH, PW), np.float32)
        r0 = hf * HH - PAD
        r1 = r0 + PH
        v0, v1 = max(r0, 0), min(r1, H)
        slab[:, v0 - r0:v1 - r0, PAD:PAD + W] = inputs[b, :, v0:v1, :]
        xs.append(slab)
    return xs, w1t, b1p, w2bc


def kernel(inputs, W1, b1, gamma, beta, mean, var, W2, b2):
    global LAST_RESULT
    inputs = np.asarray(inputs, np.float32)
    if "nc" not in _CACHE:
        _CACHE["nc"] = _build_nc()
    nc = _CACHE["nc"]

    xs, w1t, b1p, w2bc = _prep_host_inputs(
        inputs, np.asarray(W1, np.float32), np.asarray(b1, np.float32),
        np.asarray(gamma, np.float32), np.asarray(beta, np.float32),
        np.asarray(mean, np.float32), np.asarray(var, np.float32),
        np.asarray(W2, np.float32), np.asarray(b2, np.float32))

    in_maps = [{"x": xs[core], "w1t": w1t, "b1p": b1p, "w2bc": w2bc}
               for core in range(NCORES)]
    res = run_bass_kernel_spmd(nc, in_maps, list(range(NCORES)), trace=TRACE)
    LAST_RESULT = res

    out = np.empty((B, C, H, W), np.float32)
    for core in range(NCORES):
        b, hf = core // 2, core % 2
        out[b, :, hf * HH:(hf + 1) * HH, :] = res.results[core]["out"]
    return out



# revision 6
# speedup vs baseline: 1.5809x; 1.5809x over previous
"""Involution (B=4, C=256, H=W=56, K=7, G=16, reduction=4) on 8 trn2 NeuronCores.

v2 design — pixel-blocks on partitions, compact kernel values, PE-side
tap accumulation:

Sharding: 8 shards = (batch b in 0..3) x (w-half in 0..1); each core computes
a [256, 56, 28] output slab.

Per-core partition layout: p = 32*pb + 16*ch + g for pb in 0..3 (14-row
pixel blocks), ch in 0..1 (8-channel halves of a group), g in 0..15
(groups); free dims carry (c' in 0..7, rows, cols).

Pipeline per core:
  1. stage1 (PE): t_ext = [relu(bn(W1 @ x)); ones] in bf16, [65, 1568].
  2. per-tap compact kernel matmul (PE): for tap k, 4 matmuls (one per pb,
     tile_position=(0,32pb)) produce w2 psum [128, 392] = w[g, k, pix]
     replicated only 2x (for ch), NOT 16x. One small ACT copy -> bf16 SBUF.
  3. involution mul (DVE): prod = x2win(i,j) * w2sb[:, k] where the w
     operand broadcasts over c' via a stride-0 AP dim (stays in 2x mode).
  4. tap accumulation: mostly PE identity-matmuls accumulating prod into a
     f32 PSUM accumulator (start/stop flags); a few taps go to DVE/Pool
     bf16 SBUF accumulators to balance engine load; merged at the end.
"""

import numpy as np
import ml_dtypes
from contextlib import ExitStack

import concourse.bass as bass
import concourse.bacc as bacc
import concourse.tile as tile
from concourse import mybir
from concourse.bass_utils import run_bass_kernel_spmd

BF16 = ml_dtypes.bfloat16

B, C, H, W = 4, 256, 56, 56
KK, G, PAD = 7, 16, 3
Cr = 64
EPS = 1e-5
WH = W // 2               # 28 cols per w-half shard
NPIX = H * WH             # 1568 output pixels per shard
NPB = 4                   # pixel blocks (partition dim)
RB = H // NPB             # 14 rows per block
PS = RB * WH              # 392 pixels per block
XR, XC = RB + 2 * PAD, WH + 2 * PAD  # 20, 34 per-block padded window
NCORES = 8

# per-tap accumulation engine: 'PE' (psum f32), 'DVE' or 'POOL' (sbuf bf16)
ADD_ASSIGN = []
for _k in range(49):
    if _k % 8 == 3:
        ADD_ASSIGN.append('DVE')
    elif _k % 16 == 7:
        ADD_ASSIGN.append('POOL')
    else:
        ADD_ASSIGN.append('PE')

_CACHE = {}

# set by test.py to collect a hardware profile
TRACE = False
LAST_RESULT = None


def _build_nc():
    nc = bacc.Bacc("TRN2", target_bir_lowering=False, debug=False,
                   num_devices=NCORES)

    f32 = mybir.dt.float32
    bf16 = mybir.dt.bfloat16

    x2_d = nc.declare_dram_parameter("x2", [128, 8, XR, XC], bf16, isOutput=False)
    x2s_d = nc.declare_dram_parameter("x2s", [128, 8, XR, XC], bf16, isOutput=False)
    xd_d = nc.declare_dram_parameter("xd", [2, 128, NPIX], bf16, isOutput=False)
    w1t_d = nc.declare_dram_parameter("w1t", [2, 128, Cr], bf16, isOutput=False)
    b1p_d = nc.declare_dram_parameter("b1p", [Cr, 1], f32, isOutput=False)
    w2e_d = nc.declare_dram_parameter("w2e", [Cr + 1, 49, 32], bf16, isOutput=False)
    id_d = nc.declare_dram_parameter("ident", [128, 128], bf16, isOutput=False)
    out_d = nc.declare_dram_parameter("out", [128, 8, PS], f32, isOutput=True)

    PE_TAPS = [k for k in range(49) if ADD_ASSIGN[k] == 'PE']
    DVE_TAPS = [k for k in range(49) if ADD_ASSIGN[k] == 'DVE']
    POOL_TAPS = [k for k in range(49) if ADD_ASSIGN[k] == 'POOL']
    ACC_CHUNKS = [(0, 512), (512, 512), (1024, 512), (1536, 512),
                  (2048, 512), (2560, 512), (3072, 64)]

    with tile.TileContext(nc) as tc, ExitStack() as ctx:
        const = ctx.enter_context(tc.tile_pool(name="const", bufs=1))
        prodp = ctx.enter_context(tc.tile_pool(name="prod", bufs=4))
        accsb = ctx.enter_context(tc.tile_pool(name="accsb", bufs=1))
        outp = ctx.enter_context(tc.tile_pool(name="outp", bufs=1))
        psum_acc = ctx.enter_context(
            tc.tile_pool(name="psacc", bufs=1, space=bass.MemorySpace.PSUM))
        psum_w = ctx.enter_context(
            tc.tile_pool(name="psw", bufs=1, space=bass.MemorySpace.PSUM))

        # ---- constant / input loads ----
        xd_sb = const.tile([128, 2, NPIX], bf16)
        for chh in range(2):
            nc.sync.dma_start(xd_sb[:, chh, :], xd_d[chh])
        w1t_sb = const.tile([128, 2, Cr], bf16)
        for chh in range(2):
            nc.sync.dma_start(w1t_sb[:, chh, :], w1t_d[chh])
        b1p_sb = const.tile([Cr, 1], f32)
        nc.sync.dma_start(b1p_sb[:], b1p_d[:])
        w2e_sb = const.tile([Cr + 1, 49, 32], bf16)
        nc.sync.dma_start(w2e_sb[:], w2e_d[:])
        id_sb = const.tile([128, 128], bf16)
        nc.sync.dma_start(id_sb[:], id_d[:])
        x2_sb = const.tile([128, 8, XR, XC], bf16)
        nc.sync.dma_start(x2_sb[:], x2_d[:])
        x2s_sb = const.tile([128, 8, XR, XC], bf16)
        nc.sync.dma_start(x2s_sb[:], x2s_d[:])

        # ---- stage 1: t_ext = [relu(W1p @ x + b1p); ones] ----
        t_ext = const.tile([Cr + 1, NPIX], bf16)
        nc.vector.memset(t_ext[Cr:Cr + 1, :], 1.0)
        for q in range(NPB):
            pt = psum_w.tile([128, PS], f32, tag="pw")
            for chh in range(2):
                nc.tensor.matmul(pt[0:Cr, :], w1t_sb[:, chh, :],
                                 xd_sb[:, chh, q * PS:(q + 1) * PS],
                                 start=(chh == 0), stop=(chh == 1))
            nc.scalar.activation(t_ext[0:Cr, q * PS:(q + 1) * PS], pt[0:Cr, :],
                                 mybir.ActivationFunctionType.Relu,
                                 bias=b1p_sb[:], scale=1.0)

        # ---- per-tap pipeline ----
        w2sb = const.tile([128, 49, PS], bf16)
        acc_ps = psum_acc.tile([128, 3136], f32)
        accD = accsb.tile([128, 3136], bf16)
        accP = accsb.tile([128, 3136], bf16)

        prods = {}
        nD = nP = 0

        def issue_add(k):
            nonlocal nD, nP
            eng = ADD_ASSIGN[k]
            pr = prods.pop(k)
            prf = pr[:].rearrange("p a b c -> p (a b c)")
            if eng == 'PE':
                first = (k == PE_TAPS[0])
                last = (k == PE_TAPS[-1])
                for (o, n) in ACC_CHUNKS:
                    nc.tensor.matmul(acc_ps[:, o:o + n], id_sb[:],
                                     prf[:, o:o + n], start=first, stop=last,
                                     skip_group_check=True)
            elif eng == 'DVE':
                if nD == 0:
                    nc.vector.tensor_copy(accD[:], prf)
                else:
                    nc.vector.tensor_add(accD[:], accD[:], prf)
                nD += 1
            else:
                if nP == 0:
                    nc.gpsimd.tensor_copy(accP[:], prf)
                else:
                    nc.gpsimd.tensor_add(accP[:], accP[:], prf)
                nP += 1

        LAG = 2
        for k in range(49):
            i, j = k // KK, k % KK
            # stage 2: compact kernel values for tap k -> psum_w
            pw = psum_w.tile([128, PS], f32, tag="pw")
            for pb in range(NPB):
                nc.tensor.matmul(pw[32 * pb:32 * pb + 32, :],
                                 w2e_sb[:, k, :],
                                 t_ext[:, pb * PS:(pb + 1) * PS],
                                 start=True, stop=True,
                                 tile_position=(0, 32 * pb))
            nc.scalar.copy(w2sb[:, k, :], pw[:])
            # involution multiply for tap k
            wb = (w2sb[:, k, :].rearrange("p (r c) -> p r c", r=RB)
                  .unsqueeze(1).broadcast_to([128, 8, RB, WH]))
            if j % 2 == 0:
                xwin = x2_sb[:, :, i:i + RB, j:j + WH]
            else:
                xwin = x2s_sb[:, :, i:i + RB, j - 1:j - 1 + WH]
            pr = prodp.tile([128, 8, RB, WH], bf16, tag="pr")
            nc.vector.tensor_mul(pr[:], xwin, wb)
            prods[k] = pr
            # lagged accumulation keeps PE's stage-2 ahead of the adds
            if k >= LAG:
                issue_add(k - LAG)
        for k in range(49 - LAG, 49):
            issue_add(k)

        # ---- merge partial accumulators, write out ----
        of = outp.tile([128, 3136], f32)
        nc.scalar.copy(of[:], acc_ps[:])
        if DVE_TAPS and POOL_TAPS:
            nc.vector.tensor_add(accD[:], accD[:], accP[:])
        sb_parts = accD if DVE_TAPS else (accP if POOL_TAPS else None)
        if sb_parts is not None:
            nc.vector.tensor_add(of[:], of[:], sb_parts[:])
        nc.sync.dma_start(out_d[:], of[:].rearrange("p (a s) -> p a s", a=8))

    nc.compile()
    return nc


def _prep_host_inputs(inputs, W1, b1, gamma, beta, mean, var, W2, b2):
    scale = gamma / np.sqrt(var + EPS)
    shift = beta - mean * scale
    W1p = W1 * scale[:, None]
    b1p = (b1 * scale + shift).astype(np.float32).reshape(Cr, 1)
    w1t = np.ascontiguousarray(W1p.T.reshape(2, 128, Cr)).astype(BF16)

    # stage-2 lhsT: [65, 49, 32], columns (16*ch + g) duplicated over ch
    W2r = W2.reshape(G, KK * KK, Cr)       # [g, k, m]
    b2r = b2.reshape(G, KK * KK)
    w2e = np.zeros((Cr + 1, 49, 32), np.float32)
    for ch in range(2):
        w2e[0:Cr, :, 16 * ch:16 * ch + 16] = np.transpose(W2r, (2, 1, 0))
        w2e[Cr, :, 16 * ch:16 * ch + 16] = b2r.T
    w2e = w2e.astype(BF16)

    ident = np.eye(128, dtype=np.float32).astype(BF16)

    xb = np.asarray(inputs, np.float32)
    # padded: rows 3+56+3, cols 3+56+4 (extra right col for the shifted copy)
    xp = np.pad(xb, ((0, 0), (0, 0), (PAD, PAD), (PAD, PAD + 1)))

    per_core = []
    for core in range(NCORES):
        b, wh = core // 2, core % 2
        x2 = np.zeros((128, 8, XR, XC), BF16)
        x2s = np.zeros((128, 8, XR, XC), BF16)
        base = xp[b]                        # [256, 62, 63]
        c0 = wh * WH                        # global col offset of this half
        for pb in range(NPB):
            rows = slice(RB * pb, RB * pb + XR)
            blk = base[:, rows, c0:c0 + XC].astype(BF16)     # [256, 20, 34]
            blks = base[:, rows, c0 + 1:c0 + 1 + XC].astype(BF16)
            # channel -> (g, ch, c'); partition = 32*pb + 16*ch + g
            for ch in range(2):
                for g in range(G):
                    p = 32 * pb + 16 * ch + g
                    cidx = 16 * g + 8 * ch
                    x2[p] = blk[cidx:cidx + 8]
                    x2s[p] = blks[cidx:cidx + 8]
        xd = np.ascontiguousarray(
            xb[b, :, :, c0:c0 + WH].reshape(2, 128, NPIX)).astype(BF16)
        per_core.append({"x2": x2, "x2s": x2s, "xd": xd, "w1t": w1t,
                         "b1p": b1p, "w2e": w2e, "ident": ident})
    return per_core


def kernel(inputs, W1, b1, gamma, beta, mean, var, W2, b2):
    global LAST_RESULT
    inputs = np.asarray(inputs, np.float32)
    if "nc" not in _CACHE:
        _CACHE["nc"] = _build_nc()
    nc = _CACHE["nc"]

    in_maps = _prep_host_inputs(
        inputs, np.asarray(W1, np.float32), np.asarray(b1, np.float32),
        np.asarray(gamma, np.float32), np.asarray(beta, np.float32),
        np.asarray(mean, np.float32), np.asarray(var, np.float32),
        np.asarray(W2, np.float32), np.asarray(b2, np.float32))

    res = run_bass_kernel_spmd(nc, in_maps, list(range(NCORES)), trace=TRACE)
    LAST_RESULT = res

    out = np.empty((B, C, H, W), np.float32)
    for core in range(NCORES):
        b, wh = core // 2, core % 2
        o = res.results[core]["out"].reshape(4, 2, G, 8, RB, WH)
        # (pb, ch, g, c', r, c) -> channel (g, ch, c'), row (pb, r)
        o = o.transpose(2, 1, 3, 0, 4, 5).reshape(C, H, WH)
        out[b, :, :, wh * WH:(wh + 1) * WH] = o
    return out


# revision 10
# speedup vs baseline: 1.9504x; 1.2337x over previous
"""Involution (B=4, C=256, H=W=56, K=7, G=16, reduction=4) on 8 trn2 NeuronCores.

v2 design — pixel-blocks on partitions, compact kernel values, PE-side
tap accumulation:

Sharding: 8 shards = (batch b in 0..3) x (w-half in 0..1); each core computes
a [256, 56, 28] output slab.

Per-core partition layout: p = 32*pb + 16*ch + g for pb in 0..3 (14-row
pixel blocks), ch in 0..1 (8-channel halves of a group), g in 0..15
(groups); free dims carry (c' in 0..7, rows, cols).

Pipeline per core:
  1. stage1 (PE): t_ext = [relu(bn(W1 @ x)); ones] in bf16, [65, 1568].
  2. per-tap compact kernel matmul (PE): for tap k, 4 matmuls (one per pb,
     tile_position=(0,32pb)) produce w2 psum [128, 392] = w[g, k, pix]
     replicated only 2x (for ch), NOT 16x. One small ACT copy -> bf16 SBUF.
  3. involution mul (DVE): prod = x2win(i,j) * w2sb[:, k] where the w
     operand broadcasts over c' via a stride-0 AP dim (stays in 2x mode).
  4. tap accumulation: mostly PE identity-matmuls accumulating prod into a
     f32 PSUM accumulator (start/stop flags); a few taps go to DVE/Pool
     bf16 SBUF accumulators to balance engine load; merged at the end.
"""

import numpy as np
import ml_dtypes
from contextlib import ExitStack

import concourse.bass as bass
import concourse.bacc as bacc
import concourse.tile as tile
from concourse import mybir
from concourse.bass_utils import run_bass_kernel_spmd

BF16 = ml_dtypes.bfloat16

B, C, H, W = 4, 256, 56, 56
KK, G, PAD = 7, 16, 3
Cr = 64
EPS = 1e-5
WH = W // 2               # 28 cols per w-half shard
NPIX = H * WH             # 1568 output pixels per shard
NPB = 4                   # pixel blocks (partition dim)
RB = H // NPB             # 14 rows per block
PS = RB * WH              # 392 pixels per block
XR, XC = RB + 2 * PAD, WH + 2 * PAD  # 20, 34 per-block padded window
NCORES = 8

# per-tap accumulation engine: 'PE' (psum f32), 'DVE' or 'POOL' (sbuf bf16)
ADD_ASSIGN = []
for _k in range(49):
    if _k % 16 == 11:
        ADD_ASSIGN.append('DVE')
    else:
        ADD_ASSIGN.append('PE')

_CACHE = {}

# set by test.py to collect a hardware profile
TRACE = False
LAST_RESULT = None


def _build_nc():
    nc = bacc.Bacc("TRN2", target_bir_lowering=False, debug=False,
                   num_devices=NCORES)

    f32 = mybir.dt.float32
    bf16 = mybir.dt.bfloat16

    x2_d = nc.declare_dram_parameter("x2", [128, 8, XR, XC], bf16, isOutput=False)
    x2s_d = nc.declare_dram_parameter("x2s", [128, 8, XR, XC], bf16, isOutput=False)
    xd_d = nc.declare_dram_parameter("xd", [2, 128, NPIX], bf16, isOutput=False)
    w1t_d = nc.declare_dram_parameter("w1t", [2, 128, Cr], bf16, isOutput=False)
    b1p_d = nc.declare_dram_parameter("b1p", [Cr, 1], f32, isOutput=False)
    w2e_d = nc.declare_dram_parameter("w2e", [Cr + 1, 49, 32], bf16, isOutput=False)
    id_d = nc.declare_dram_parameter("ident", [128, 128], bf16, isOutput=False)
    out_d = nc.declare_dram_parameter("out", [128, 8, PS], f32, isOutput=True)

    PE_TAPS = [k for k in range(49) if ADD_ASSIGN[k] == 'PE']
    DVE_TAPS = [k for k in range(49) if ADD_ASSIGN[k] == 'DVE']
    POOL_TAPS = [k for k in range(49) if ADD_ASSIGN[k] == 'POOL']
    # 7 uniform 448-col chunks; acc tile is [128, 7, 512] so each chunk
    # starts at a psum bank boundary (matmul dst must stay within a bank)
    ACC_CHUNKS = [(c, 448) for c in range(7)]

    with tile.TileContext(nc) as tc, ExitStack() as ctx:
        const = ctx.enter_context(tc.tile_pool(name="const", bufs=1))
        prodp = ctx.enter_context(tc.tile_pool(name="prod", bufs=4))
        accsb = ctx.enter_context(tc.tile_pool(name="accsb", bufs=1))
        outp = ctx.enter_context(tc.tile_pool(name="outp", bufs=1))
        psum_acc = ctx.enter_context(
            tc.tile_pool(name="psacc", bufs=1, space=bass.MemorySpace.PSUM))
        psum_w = ctx.enter_context(
            tc.tile_pool(name="psw", bufs=1, space=bass.MemorySpace.PSUM))

        # ---- constant / input loads ----
        xd_sb = const.tile([128, 2, NPIX], bf16)
        for chh in range(2):
            nc.sync.dma_start(xd_sb[:, chh, :], xd_d[chh])
        w1t_sb = const.tile([128, 2, Cr], bf16)
        for chh in range(2):
            nc.sync.dma_start(w1t_sb[:, chh, :], w1t_d[chh])
        b1p_sb = const.tile([Cr, 1], f32)
        nc.sync.dma_start(b1p_sb[:], b1p_d[:])
        w2e_sb = const.tile([Cr + 1, 49, 32], bf16)
        nc.sync.dma_start(w2e_sb[:], w2e_d[:])
        id_sb = const.tile([128, 128], bf16)
        nc.sync.dma_start(id_sb[:], id_d[:])
        x2_sb = const.tile([128, 8, XR, XC], bf16)
        nc.sync.dma_start(x2_sb[:], x2_d[:])
        x2s_sb = const.tile([128, 8, XR, XC], bf16)
        nc.sync.dma_start(x2s_sb[:], x2s_d[:])

        # ---- stage 1: t_ext = [relu(W1p @ x + b1p); ones] ----
        t_ext = const.tile([Cr + 1, NPIX], bf16)
        nc.vector.memset(t_ext[Cr:Cr + 1, :], 1.0)
        for q in range(NPB):
            pt = psum_w.tile([128, PS], f32, tag="pw")
            for chh in range(2):
                nc.tensor.matmul(pt[0:Cr, :], w1t_sb[:, chh, :],
                                 xd_sb[:, chh, q * PS:(q + 1) * PS],
                                 start=(chh == 0), stop=(chh == 1))
            nc.scalar.activation(t_ext[0:Cr, q * PS:(q + 1) * PS], pt[0:Cr, :],
                                 mybir.ActivationFunctionType.Relu,
                                 bias=b1p_sb[:], scale=1.0)

        # ---- per-tap pipeline ----
        w2sb = const.tile([128, 49, PS], bf16)
        acc_ps = psum_acc.tile([128, 7, 512], f32)
        accD = accsb.tile([128, 3136], bf16)

        prods = {}
        nD = 0

        def issue_add(k):
            nonlocal nD
            eng = ADD_ASSIGN[k]
            pr = prods.pop(k)
            prf = pr[:].rearrange("p a b c -> p (a b c)")
            if eng == 'PE':
                first = (k == PE_TAPS[0])
                last = (k == PE_TAPS[-1])
                for (c, n) in ACC_CHUNKS:
                    nc.tensor.matmul(acc_ps[:, c, 0:n], id_sb[:],
                                     prf[:, c * n:(c + 1) * n],
                                     start=first, stop=last,
                                     skip_group_check=True)
            else:
                if nD == 0:
                    nc.vector.tensor_copy(accD[:], prf)
                else:
                    nc.vector.tensor_add(accD[:], accD[:], prf)
                nD += 1

        LAG = 2
        for k in range(49):
            i, j = k // KK, k % KK
            # stage 2: compact kernel values for tap k -> psum_w
            pw = psum_w.tile([128, PS], f32, tag="pw")
            for pb in range(NPB):
                nc.tensor.matmul(pw[32 * pb:32 * pb + 32, :],
                                 w2e_sb[:, k, :],
                                 t_ext[:, pb * PS:(pb + 1) * PS],
                                 start=True, stop=True,
                                 tile_position=(0, 32 * pb))
            nc.scalar.copy(w2sb[:, k, :], pw[:])
            # involution multiply for tap k
            wb = (w2sb[:, k, :].rearrange("p (r c) -> p r c", r=RB)
                  .unsqueeze(1).broadcast_to([128, 8, RB, WH]))
            if j % 2 == 0:
                xwin = x2_sb[:, :, i:i + RB, j:j + WH]
            else:
                xwin = x2s_sb[:, :, i:i + RB, j - 1:j - 1 + WH]
            pr = prodp.tile([128, 8, RB, WH], bf16, tag="pr")
            nc.vector.tensor_mul(pr[:], xwin, wb)
            prods[k] = pr
            # lagged accumulation keeps PE's stage-2 ahead of the adds
            if k >= LAG:
                issue_add(k - LAG)
        for k in range(49 - LAG, 49):
            issue_add(k)

        # ---- merge partial accumulators, write out ----
        of = outp.tile([128, 3136], f32)
        ofv = of[:].rearrange("p (c n) -> p c n", c=7)
        nc.scalar.copy(ofv, acc_ps[:, :, 0:448])
        if DVE_TAPS:
            nc.vector.tensor_add(of[:], of[:], accD[:])
        nc.sync.dma_start(out_d[:], of[:].rearrange("p (a s) -> p a s", a=8))

    nc.compile()
    return nc


def _prep_host_inputs(inputs, W1, b1, gamma, beta, mean, var, W2, b2):
    scale = gamma / np.sqrt(var + EPS)
    shift = beta - mean * scale
    W1p = W1 * scale[:, None]
    b1p = (b1 * scale + shift).astype(np.float32).reshape(Cr, 1)
    w1t = np.ascontiguousarray(W1p.T.reshape(2, 128, Cr)).astype(BF16)

    # stage-2 lhsT: [65, 49, 32], columns (16*ch + g) duplicated over ch
    W2r = W2.reshape(G, KK * KK, Cr)       # [g, k, m]
    b2r = b2.reshape(G, KK * KK)
    w2e = np.zeros((Cr + 1, 49, 32), np.float32)
    for ch in range(2):
        w2e[0:Cr, :, 16 * ch:16 * ch + 16] = np.transpose(W2r, (2, 1, 0))
        w2e[Cr, :, 16 * ch:16 * ch + 16] = b2r.T
    w2e = w2e.astype(BF16)

    ident = np.eye(128, dtype=np.float32).astype(BF16)

    xb = np.asarray(inputs, np.float32)
    # padded: rows 3+56+3, cols 3+56+4 (extra right col for the shifted copy)
    xp = np.pad(xb, ((0, 0), (0, 0), (PAD, PAD), (PAD, PAD + 1)))

    per_core = []
    for core in range(NCORES):
        b, wh = core // 2, core % 2
        x2 = np.zeros((128, 8, XR, XC), BF16)
        x2s = np.zeros((128, 8, XR, XC), BF16)
        base = xp[b]                        # [256, 62, 63]
        c0 = wh * WH                        # global col offset of this half
        for pb in range(NPB):
            rows = slice(RB * pb, RB * pb + XR)
            blk = base[:, rows, c0:c0 + XC].astype(BF16)     # [256, 20, 34]
            blks = base[:, rows, c0 + 1:c0 + 1 + XC].astype(BF16)
            # channel -> (g, ch, c'); partition = 32*pb + 16*ch + g
            for ch in range(2):
                for g in range(G):
                    p = 32 * pb + 16 * ch + g
                    cidx = 16 * g + 8 * ch
                    x2[p] = blk[cidx:cidx + 8]
                    x2s[p] = blks[cidx:cidx + 8]
        xd = np.ascontiguousarray(
            xb[b, :, :, c0:c0 + WH].reshape(2, 128, NPIX)).astype(BF16)
        per_core.append({"x2": x2, "x2s": x2s, "xd": xd, "w1t": w1t,
                         "b1p": b1p, "w2e": w2e, "ident": ident})
    return per_core


def kernel(inputs, W1, b1, gamma, beta, mean, var, W2, b2):
    global LAST_RESULT
    inputs = np.asarray(inputs, np.float32)
    if "nc" not in _CACHE:
        _CACHE["nc"] = _build_nc()
    nc = _CACHE["nc"]

    in_maps = _prep_host_inputs(
        inputs, np.asarray(W1, np.float32), np.asarray(b1, np.float32),
        np.asarray(gamma, np.float32), np.asarray(beta, np.float32),
        np.asarray(mean, np.float32), np.asarray(var, np.float32),
        np.asarray(W2, np.float32), np.asarray(b2, np.float32))

    res = run_bass_kernel_spmd(nc, in_maps, list(range(NCORES)), trace=TRACE)
    LAST_RESULT = res

    out = np.empty((B, C, H, W), np.float32)
    for core in range(NCORES):
        b, wh = core // 2, core % 2
        o = res.results[core]["out"].reshape(4, 2, G, 8, RB, WH)
        # (pb, ch, g, c', r, c) -> channel (g, ch, c'), row (pb, r)
        o = o.transpose(2, 1, 3, 0, 4, 5).reshape(C, H, WH)
        out[b, :, :, wh * WH:(wh + 1) * WH] = o
    return out
